# revision 28
# baseline (speedup 1.0000x reference)
"""Cross-modal attention kernel for Trainium2 (8 NeuronCores, data-parallel over batch).

Exact weight folds (host, weights only):
  Wqk = (Wq*s) @ Wk^T,  Wvo = Wv @ Wo,  wkbq = Wk @ (bq*s),  bo' = bo + bv @ Wo
  scores^T = Wqk^T-chain: scoresT[k,q] = key @ Wqk^T @ query^T + bqk[k]
  out      = ((P @ key) @ Wvo) / denom  (+ bo' + query residual, added host-side)

All GEMMs run as fp8-e4m3 DoubleRow matmuls (two 128-row contraction slices
per instruction, 2 MACs/cell/cycle).  Operands are pair-packed host-side as
[128, 2, free] tiles (partition p, pair-slot ab covers contraction rows
g*256 + ab*128 + p), so no on-chip transposes are needed:
  qt8  = query^T  pair-packed over D      (x 512/8 net scale via copies)
  kt8  = key^T    pair-packed over DK     (used by scores and the bqk bias)
  k8   = key      pair-packed over LK     (V side of P @ key)
Per 1024-wide q-tile:
  t1t   = Wqk^T @ queryT     [DK, 1024]  psum -> fp8 (*1/8; net t1t = 64*T1)
  scT   = keyT^T @ t1t       [LK, 1024]  psum = 64*scores
  PT    = exp(scT/64 + bqk)  fp8, 1024-wide Act instrs, exact per-k bias
  denom = PT^T @ ones        DoubleRow, out-free=1 (nearly free on PE)
  attT  = key^T-stationary @ PT  [DK, 1024] psum -> fp8 (*1/32)
  out   = (attT^T @ (32*Wvo)) * (1/denom)  -> fp16 attention-only output
Residual (query + bo') is added host-side in fp32."""

import numpy as np
import ml_dtypes

import concourse.bacc as bacc
import concourse.tile as tile
import concourse.mybir as mybir
from concourse.bass_utils import run_bass_kernel_spmd

B, LQ, LK = 8, 2048, 2048
D, DK, H = 1024, 512, 1024
SCALE = 1.0 / np.sqrt(H)
F32, F16, F8 = mybir.dt.float32, mybir.dt.float16, mybir.dt.float8e4
F8NP = ml_dtypes.float8_e4m3
AF = mybir.ActivationFunctionType
DRM = mybir.MatmulPerfMode.DoubleRow

NCORES = 8
QT_W = 1024
NQT = LQ // QT_W      # 2
WQK_S = 512.0         # host scale on Wqk upload
ALPHA = 64.0          # net scale on t1t fp8 (= 64 * T1)
WKBQ_S = 512.0        # host scale on wkbq upload
GAMMA = 1.0 / 32.0    # scale on attT fp8 copy
WVO_S = 32.0          # host scale on Wvo upload (GAMMA * WVO_S = 1)


def _emit(nc, tc, io):
    """Software-pipelined emission. Engines execute in program order, so the
    emission order IS the schedule:
      [loads | warmup] t1t(0) -> scores/exp(0) backbone
      under exp(0): t1t(1); then dn(0)
      middle: scores/exp(1) interleaved with att(0) (kc<8) and out(0) (kc>=8)
      tail: dn(1), att(1), out(1)
    PSUM: "sc" 2x[128,1024] (4 banks) + "wk" 3x[128,512] (3 banks, shared by
    t1t/att/out fills) + "dn" [128,16] (1 bank) = 8 banks."""
    with tc.tile_pool(name="ps", bufs=1, space="PSUM") as ps, \
         tc.tile_pool(name="pers", bufs=1) as pers, \
         tc.tile_pool(name="wp", bufs=1) as wp:
        # ---- persistent fp8 operands (all pair-packed [128, 2, free]) ----
        qt8 = [pers.tile([128, 2, 2048], F8, tag=f"qt8_{i}", name=f"qt8_{i}")
               for i in range(4)]
        kt8 = [pers.tile([128, 2, 2048], F8, tag=f"kt8_{i}", name=f"kt8_{i}")
               for i in range(2)]
        k8 = [pers.tile([128, 2, 512], F8, tag=f"k8_{j}", name=f"k8_{j}")
              for j in range(8)]
        wqk8 = [pers.tile([128, 2, 512], F8, tag=f"wqk8_{i}", name=f"wqk8_{i}")
                for i in range(4)]
        wvo8 = [pers.tile([128, 2, 1024], F8, tag=f"wvo8_{i}", name=f"wvo8_{i}")
                for i in range(2)]
        wkbq8 = pers.tile([128, 2, 16], F8, tag="wkbq8", name="wkbq8")
        ones8 = pers.tile([128, 2, 16], F8, tag="ones8", name="ones8")
        zero8 = pers.tile([128, 2, 512], F8, tag="zero8", name="zero8")
        bqk_sb = pers.tile([128, 16], F32, tag="bqk_sb", name="bqk_sb")

        nc.vector.memset(ones8[:], 1.0)
        nc.vector.memset(zero8[:], 0.0)

        # loads, in first-use order; qt8/kt8 split so the columns qtile-0 and
        # the early kc's need arrive first.  Late consumers via gpsimd/SWDGE.
        # spread DMA issue across the four idle sequencers at startup
        for i in range(4):
            nc.sync.dma_start(out=wqk8[i][:], in_=io["wqk8"][i])
        for i in range(4):
            nc.sync.dma_start(out=qt8[i][:, :, 0:512],
                              in_=io["qt8"][i][:, :, 0:512])
        nc.sync.dma_start(out=wkbq8[:], in_=io["wkbq8"][:])
        for i in range(4):
            nc.sync.dma_start(out=qt8[i][:, :, 512:1024],
                              in_=io["qt8"][i][:, :, 512:1024])
        for i in range(2):
            nc.sync.dma_start(out=kt8[i][:, :, 0:1024],
                              in_=io["kt8"][i][:, :, 0:1024])
        for i in range(4):
            nc.sync.dma_start(out=qt8[i][:, :, 1024:2048],
                              in_=io["qt8"][i][:, :, 1024:2048])
        for i in range(2):
            nc.sync.dma_start(out=kt8[i][:, :, 1024:2048],
                              in_=io["kt8"][i][:, :, 1024:2048])
        for j in range(8):
            nc.gpsimd.dma_start(out=k8[j][:], in_=io["k8"][j])
        for i in range(2):
            nc.gpsimd.dma_start(out=wvo8[i][:], in_=io["wvo8"][i])

        # PE warm-up: keep the p-state ramp clock running during DMA fill.
        for w in range(6):
            du = ps.tile([128, 512], F32, tag="wk", bufs=3, name=f"du{w}")
            nc.tensor.matmul(du[:], zero8[:, :, 0:128], zero8[:],
                             start=True, stop=True, perf_mode=DRM)

        # bqk[k] = key @ (Wk @ bq * s) — exact per-k softmax bias, on device.
        # Two groups (kc halves) so the first copy only waits on kt8 half A.
        bqkp = ps.tile([128, 16], F32, tag="dn", name="bqkp")

        def bqk_half(h):
            for kc in range(8 * h, 8 * h + 8):
                for i in range(2):
                    nc.tensor.matmul(bqkp[:, kc:kc + 1],
                                     kt8[i][:, :, kc * 128:(kc + 1) * 128],
                                     wkbq8[:, :, 0:1],
                                     start=(kc == 8 * h and i == 0),
                                     stop=(kc == 8 * h + 7 and i == 1),
                                     perf_mode=DRM, skip_group_check=True)
            nc.vector.tensor_scalar_mul(bqk_sb[:, 8 * h:8 * h + 8],
                                        bqkp[:, 8 * h:8 * h + 8], 1.0 / WKBQ_S)

        # ---- phase helpers ----------------------------------------------
        def t1t_round(qt, t1tp, dkc, qh, on_act=False):
            t1 = ps.tile([128, 512], F32, tag="wk", bufs=3, name=f"t1_{qt}_{dkc}{qh}")
            q0 = qt * 1024 + qh * 512
            for i in range(4):
                nc.tensor.matmul(
                    t1[:], wqk8[i][:, :, dkc * 128:(dkc + 1) * 128],
                    qt8[i][:, :, q0:q0 + 512],
                    start=(i == 0), stop=(i == 3), perf_mode=DRM)
            dst = t1tp[dkc // 2][:, dkc % 2, qh * 512:(qh + 1) * 512]
            if on_act:
                nc.scalar.mul(dst, t1[:], ALPHA / WQK_S)
            else:
                nc.vector.tensor_scalar_mul(dst, t1[:], ALPHA / WQK_S)

        def sc_exp(qt, t1tp, ptt, kc):
            sc = ps.tile([128, 1024], F32, tag="sc", bufs=2,
                         name=f"sc_{qt}_{kc}")
            for qh in range(2):
                for i in range(2):
                    nc.tensor.matmul(
                        sc[:, qh * 512:(qh + 1) * 512],
                        kt8[i][:, :, kc * 128:(kc + 1) * 128],
                        t1tp[i][:, :, qh * 512:(qh + 1) * 512],
                        start=(i == 0), stop=(i == 1), perf_mode=DRM)
            nc.scalar.activation(ptt[kc // 2][:, kc % 2, :], sc[:],
                                 AF.Exp, bias=bqk_sb[:, kc:kc + 1],
                                 scale=1.0 / ALPHA)

        def dn_batch(dn, ptt, j):
            # j-outer batches: emit right after exp(qt, 2j+1) so the group
            # progresses during the exp chain with no PE stall.
            for qc in range(8):
                nc.tensor.matmul(dn[:, qc:qc + 1],
                                 ptt[j][:, :, qc * 128:(qc + 1) * 128],
                                 ones8[:, :, 0:1],
                                 start=(j == 0 and qc == 0),
                                 stop=(j == 7 and qc == 7),
                                 perf_mode=DRM, skip_group_check=True)

        def dn_finish(qt, dn):
            recip = wp.tile([128, 8], F32, tag="recip", bufs=2,
                            name=f"recip{qt}")
            nc.vector.reciprocal(recip[:], dn[:, 0:8])
            return recip

        def att_round(qt, ptt, att8p, r, on_act):
            dkc, qh = r // 2, r % 2
            at = ps.tile([128, 512], F32, tag="wk", bufs=3, name=f"at_{qt}_{r}")
            for j in range(8):
                nc.tensor.matmul(
                    at[:], k8[j][:, :, dkc * 128:(dkc + 1) * 128],
                    ptt[j][:, :, qh * 512:(qh + 1) * 512],
                    start=(j == 0), stop=(j == 7), perf_mode=DRM)
            dst = att8p[dkc // 2][:, dkc % 2, qh * 512:(qh + 1) * 512]
            if on_act:
                nc.scalar.mul(dst, at[:], GAMMA)
            else:
                nc.vector.tensor_scalar_mul(dst, at[:], GAMMA)

        def out_round(qt, att8p, recip, outsb, r, on_act):
            qc, dh = r // 2, r % 2
            ou = ps.tile([128, 512], F32, tag="wk", bufs=3, name=f"ou_{qt}_{r}")
            for i in range(2):
                nc.tensor.matmul(
                    ou[:], att8p[i][:, :, qc * 128:(qc + 1) * 128],
                    wvo8[i][:, :, dh * 512:(dh + 1) * 512],
                    start=(i == 0), stop=(i == 1), perf_mode=DRM)
            dst = outsb[:, qc, dh * 512:(dh + 1) * 512]
            if on_act:
                nc.scalar.mul(dst, ou[:], recip[:, qc:qc + 1])
            else:
                nc.vector.tensor_scalar_mul(dst, ou[:], recip[:, qc:qc + 1])

        def out_wide(qt, att8p, recip, outsb, qc, on_act):
            # tail variant: borrow the (now idle) "sc" pool for a 2-bank psum
            # and do one 1024-wide normalize copy per q-chunk.
            ou = ps.tile([128, 1024], F32, tag="sc", bufs=2,
                         name=f"ouw_{qt}_{qc}")
            for dh in range(2):
                for i in range(2):
                    nc.tensor.matmul(
                        ou[:, dh * 512:(dh + 1) * 512],
                        att8p[i][:, :, qc * 128:(qc + 1) * 128],
                        wvo8[i][:, :, dh * 512:(dh + 1) * 512],
                        start=(i == 0), stop=(i == 1), perf_mode=DRM)
            dst = outsb[:, qc, :]
            if on_act:
                nc.scalar.mul(dst, ou[:], recip[:, qc:qc + 1])
            else:
                nc.vector.tensor_scalar_mul(dst, ou[:], recip[:, qc:qc + 1])

        def alloc_qt(qt):
            t1tp = [wp.tile([128, 2, 1024], F8, tag=f"t1tp{i}", bufs=2,
                            name=f"t1tp{i}_{qt}") for i in range(2)]
            ptt = [wp.tile([128, 2, 1024], F8, tag=f"ptt{j}", bufs=2,
                           name=f"ptt{j}_{qt}") for j in range(8)]
            att8p = [wp.tile([128, 2, 1024], F8, tag=f"att8p{i}", bufs=2,
                             name=f"att8p{i}_{qt}") for i in range(2)]
            outsb = wp.tile([128, 8, 1024], F16, tag="outsb", bufs=2,
                            name=f"outsb{qt}")
            return t1tp, ptt, att8p, outsb

        # ---- pipelined schedule -----------------------------------------
        t1tp0, ptt0, att8p0, outsb0 = alloc_qt(0)
        t1tp1, ptt1, att8p1, outsb1 = alloc_qt(1)

        # qh-major so the first 4 rounds only need the qt8 [0:512] quarters;
        # copies alternate DVE/ACT (both idle during startup)
        for r in range(4):
            t1t_round(0, t1tp0, r, 0, on_act=(r % 2 == 1))
        bqk_half(0)                               # needs only kt8 half A
        for r in range(4):
            t1t_round(0, t1tp0, r, 1, on_act=(r % 2 == 0))
        bqk_half(1)

        # region 1: exp(0) backbone; t1t(1) + denom(0) slotted between the
        # sc fills so PE never blocks in-order on the sc-buffer round trip.
        dn0 = ps.tile([128, 16], F32, tag="dn", name="dn0")
        for kc in range(16):
            sc_exp(0, t1tp0, ptt0, kc)
            if kc % 2 == 0:
                t1t_round(1, t1tp1, kc // 4, (kc // 2) % 2)
            elif kc >= 3:
                dn_batch(dn0, ptt0, (kc - 3) // 2)   # trails exp by 2
        dn_batch(dn0, ptt0, 7)
        recip0 = dn_finish(0, dn0)

        # region 2: exp(1) backbone; att(0) then out(0) slotted between
        dn1 = ps.tile([128, 16], F32, tag="dn", name="dn1")
        for kc in range(16):
            sc_exp(1, t1tp1, ptt1, kc)
            if kc % 2 == 1 and kc >= 3:
                dn_batch(dn1, ptt1, (kc - 3) // 2)   # trails exp by 2
            if kc < 8:
                att_round(0, ptt0, att8p0, kc, on_act=False)
            else:
                out_round(0, att8p0, recip0, outsb0, 2 * (kc - 8), on_act=False)
                out_round(0, att8p0, recip0, outsb0, 2 * (kc - 8) + 1,
                          on_act=False)
        nc.sync.dma_start(out=io["attn16"][:, 0], in_=outsb0[:])
        dn_batch(dn1, ptt1, 7)
        recip1 = dn_finish(1, dn1)

        # tail: att(1) on wk pool, out(1) 1024-wide on the freed sc pool
        for r in range(8):
            att_round(1, ptt1, att8p1, r, on_act=(r % 2 == 1))
        for qc in range(8):
            if qc % 2 == 0:
                out_wide(1, att8p1, recip1, outsb1, qc, on_act=False)
            else:
                out_round(1, att8p1, recip1, outsb1, 2 * qc, on_act=True)
                out_round(1, att8p1, recip1, outsb1, 2 * qc + 1, on_act=True)
            if qc % 2 == 1:
                nc.sync.dma_start(out=io["attn16"][:, 1, qc - 1:qc + 1],
                                  in_=outsb1[:, qc - 1:qc + 1])


_NC = None


def _build():
    global _NC
    if _NC is not None:
        return _NC
    nc = bacc.Bacc("TRN2", target_bir_lowering=False, debug=False,
                   num_devices=NCORES)
    io = {}
    io["qt8"] = nc.dram_tensor("qt8", [4, 128, 2, 2048], F8,
                               kind="ExternalInput").ap()
    io["kt8"] = nc.dram_tensor("kt8", [2, 128, 2, 2048], F8,
                               kind="ExternalInput").ap()
    io["k8"] = nc.dram_tensor("k8", [8, 128, 2, 512], F8,
                              kind="ExternalInput").ap()
    io["wqk8"] = nc.dram_tensor("wqk8", [4, 128, 2, 512], F8,
                                kind="ExternalInput").ap()
    io["wvo8"] = nc.dram_tensor("wvo8", [2, 128, 2, 1024], F8,
                                kind="ExternalInput").ap()
    io["wkbq8"] = nc.dram_tensor("wkbq8", [128, 2, 16], F8,
                                 kind="ExternalInput").ap()
    io["attn16"] = nc.dram_tensor("attn16", [128, NQT, 8, 1024], F16,
                                  kind="ExternalOutput").ap()
    with tile.TileContext(nc) as tc:
        _emit(nc, tc, io)
    nc.compile()
    _NC = nc
    return nc


def _pack(m, s):
    """[R, C] -> [R//256, 128, 2, C] fp8: row g*256 + ab*128 + p -> [g,p,ab]."""
    r, c = m.shape
    return np.ascontiguousarray(
        np.clip(m.reshape(r // 256, 2, 128, c).transpose(0, 2, 1, 3) * s,
                -240.0, 240.0).astype(F8NP))


def _prep_shared(Wq, bq, Wk, bk, Wv, bv, Wo, bo):
    f32 = np.float32
    Wq, Wk = np.asarray(Wq, f32), np.asarray(Wk, f32)
    Wv, Wo = np.asarray(Wv, f32), np.asarray(Wo, f32)
    bq, bo, bv = np.asarray(bq, f32), np.asarray(bo, f32), np.asarray(bv, f32)
    wqk = (Wq * SCALE) @ Wk.T                      # [D, DK]
    wvo = Wv @ Wo                                  # [DK, D]
    wkbq = (Wk @ (bq * SCALE)) * WKBQ_S            # [DK]
    wkbq_p = np.zeros((128, 2, 16), f32)
    wkbq_p[:, :, 0:2] = wkbq.reshape(2, 2, 128).transpose(2, 1, 0)
    return {
        "wqk8": _pack(wqk, WQK_S),
        "wvo8": _pack(wvo, WVO_S),
        "wkbq8": np.clip(wkbq_p, -240.0, 240.0).astype(F8NP),
    }, (bo + bv @ Wo).astype(f32)


def kernel(query, key, Wq, bq, Wk, bk, Wv, bv, Wo, bo):
    nc = _build()
    shared, bo_full = _prep_shared(Wq, bq, Wk, bk, Wv, bv, Wo, bo)
    query = np.asarray(query, np.float32)
    key = np.asarray(key, np.float32)
    in_maps = []
    for c in range(NCORES):
        in_maps.append({
            "qt8": _pack(np.ascontiguousarray(query[c].T), 1.0),
            "kt8": _pack(np.ascontiguousarray(key[c].T), 1.0),
            "k8": _pack(key[c], 1.0),
            **shared,
        })
    res = run_bass_kernel_spmd(nc, in_maps, core_ids=list(range(NCORES)))
    out = np.empty((B, LQ, D), np.float32)
    for c in range(NCORES):
        attn = np.asarray(res.results[c]["attn16"]).astype(np.float32)
        attn = attn.transpose(1, 2, 0, 3).reshape(LQ, D)
        out[c] = query[c] + attn + bo_full
    return out
